# revision 1
# baseline (speedup 1.0000x reference)
"""MultiHeadAttention Trainium2 kernel.

Full inputs -> full output. Sharding: 8 cores = (batch b in 0..3) x (query
half in 0..1). Each core computes attention for its 1024 query rows of batch
b over all 2048 keys of batch b (K/V projections duplicated across the pair
of cores sharing a batch), then applies the output projection for its rows.
Outputs are disjoint row-slices of (B, S, D); host assembly is a pure concat.

All matmuls run in float32r (fp32 data at ~1 cycle/row, ~2^-12 rounding).

  phase A: transpose x row-blocks via PE-identity transposes, project
           Q^T [d,s], K^T [d,s], V [s,d] (V augmented with a ones column so
           the softmax denominator falls out of the ctx matmul); weight
           matrices stream per 128-row chunk; rank-1 matmuls add bq/bk.
  phase B: software-pipelined (head, query-block) iterations: logits^T
           [sk,sq] = K_h^T.T @ Q_h^T, P = exp(0.125*logits + mask*(-1e9))
           on the scalar engine, interleaved on the PE with the previous
           iteration's ctx matmuls; raw ctx/denominator rows go to DRAM.
  phase C: one batched reciprocal over all 32 denominator rows, PE
           broadcast of the recips, normalize raw ctx^T, then
           out = ctx @ wo + bo' where bo' = bo + bv @ wo (host-folded).
"""

import numpy as np

import concourse.bass as bass
import concourse.mybir as mybir
import concourse.tile as tile
from concourse import bacc
from concourse.bass_utils import run_bass_kernel_spmd

f32 = mybir.dt.float32
f32r = mybir.dt.float32r

B, S, D, H, DH = 4, 2048, 1024, 16, 64
SQ = S // 2          # query rows per core
N_CORES = 8
Exp = mybir.ActivationFunctionType.Exp

KC = D // 128        # 8 contraction chunks
SKC = S // 128       # 16 key chunks
NT = 2 * H           # 32 pipelined iterations (head, sqb)


def _build():
    nc = bacc.Bacc(None, target_bir_lowering=False)

    xq = nc.dram_tensor("xq", [D, SQ], f32r, kind="ExternalInput")   # query^T
    xk = nc.dram_tensor("xk", [D, S], f32r, kind="ExternalInput")    # key^T
    xv = nc.dram_tensor("xv", [D, S], f32r, kind="ExternalInput")    # value^T
    wq = nc.dram_tensor("wq", [D, D], f32r, kind="ExternalInput")
    wk = nc.dram_tensor("wk", [D, D], f32r, kind="ExternalInput")
    wv = nc.dram_tensor("wv", [D, D], f32r, kind="ExternalInput")
    wo = nc.dram_tensor("wo", [D, D], f32r, kind="ExternalInput")
    b3 = nc.dram_tensor("b3", [65, D], f32r, kind="ExternalInput")    # bq/bk/bo' at rows 0/32/64
    mb = nc.dram_tensor("mb", [128, SKC], f32, kind="ExternalInput")  # mask*-1e9 [p, chunk]
    one = nc.dram_tensor("one", [128, 512], f32r, kind="ExternalInput")
    sel = nc.dram_tensor("sel", [NT, 2, KC, 128], f32r, kind="ExternalInput")
    out = nc.dram_tensor("out", [SQ, D], f32, kind="ExternalOutput")

    with tile.TileContext(nc) as tc:
        _emit(nc, tc, xq, xk, xv, wq, wk, wv, wo, b3, mb, one, sel, out)
    nc.finalize()
    return nc


def _emit(nc, tc, xq, xk, xv, wq, wk, wv, wo, b3, mb, one, sel, out):
    from contextlib import ExitStack

    with ExitStack() as ctx:
        consts = ctx.enter_context(tc.tile_pool(name="consts", bufs=1))
        wpool = ctx.enter_context(tc.tile_pool(name="wpool", bufs=9))
        xtp = ctx.enter_context(tc.tile_pool(name="xtp", bufs=2))
        xtp2 = ctx.enter_context(tc.tile_pool(name="xtp2", bufs=2))
        qts = ctx.enter_context(tc.tile_pool(name="qts", bufs=2))
        kts = ctx.enter_context(tc.tile_pool(name="kts", bufs=2))
        vas = ctx.enter_context(tc.tile_pool(name="vas", bufs=2))
        ptp = ctx.enter_context(tc.tile_pool(name="ptp", bufs=10))
        stg = ctx.enter_context(tc.tile_pool(name="stg", bufs=2))
        cns2 = ctx.enter_context(tc.tile_pool(name="cns2", bufs=2))
        psA = ctx.enter_context(tc.tile_pool(name="psA", bufs=2, space="PSUM"))
        psX = ctx.enter_context(tc.tile_pool(name="psX", bufs=2, space="PSUM"))
        psC = ctx.enter_context(tc.tile_pool(name="psC", bufs=2, space="PSUM"))
        dram = ctx.enter_context(tc.tile_pool(name="dram", bufs=1, space="DRAM"))

        ktd = dram.tile([D, S], f32r)                 # K^T
        vad = dram.tile([SKC, 128, H, DH + 1], f32r)  # V augmented with ones col
        qtd = dram.tile([D, SQ], f32r)                # Q^T
        crd = dram.tile([D, SQ], f32r)                # raw (unnormalized) ctx^T
        dnd = dram.tile([NT, 512], f32r)              # denominator rows

        ones = consts.tile([128, 512], f32r)
        nc.sync.dma_start(ones, one[:])
        b3_sb = consts.tile([65, D], f32r)
        nc.sync.dma_start(b3_sb, b3[:])
        mb_sb = consts.tile([128, SKC], f32)
        nc.sync.dma_start(mb_sb, mb[:])
        sel_sb = consts.tile([NT, 2, KC, 128], f32r)
        nc.sync.dma_start(sel_sb, sel[:])

        def load_w(w_dram):
            chunks = []
            for kc in range(KC):
                wt = wpool.tile([128, D], f32r, tag="w", name="wt")
                nc.sync.dma_start(wt, w_dram[kc * 128:(kc + 1) * 128, :])
                chunks.append(wt)
            return chunks

        # ================= phase A =================
        def project_T(w_ch, brow, x_dram, blk, dst_dram):
            """One 512-row block of a transposed projection -> dst_dram."""
            xT = xtp.tile([128, KC, 512], f32r, tag="xT", name="xT")
            nc.sync.dma_start(xT, x_dram[:, blk * 512:(blk + 1) * 512]
                              .rearrange("(ko p) s -> p ko s", p=128))
            for dc2 in range(KC // 2):
                ps = psA.tile([128, 1024], f32, tag="psA", name="ps")
                for half in range(2):
                    dc = dc2 * 2 + half
                    ph = ps[:, half * 512:(half + 1) * 512]
                    for kc in range(KC):
                        nc.tensor.matmul(ph, lhsT=w_ch[kc][:, dc * 128:(dc + 1) * 128],
                                         rhs=xT[:, kc, :], start=(kc == 0), stop=False)
                    nc.tensor.matmul(ph, lhsT=b3_sb[brow:brow + 1, dc * 128:(dc + 1) * 128],
                                     rhs=ones[brow:brow + 1, 0:512], start=False, stop=True)
                st_t = stg.tile([128, 1024], f32r, tag="stg", name="st_t")
                nc.vector.tensor_copy(st_t, ps)
                for half in range(2):
                    dc = dc2 * 2 + half
                    nc.sync.dma_start(
                        dst_dram[dc * 128:(dc + 1) * 128, blk * 512:(blk + 1) * 512],
                        st_t[:, half * 512:(half + 1) * 512])

        wq_ch = load_w(wq)
        for sqb in range(2):
            project_T(wq_ch, 0, xq, sqb, qtd)

        wk_ch = load_w(wk)
        for skb in range(4):
            project_T(wk_ch, 32, xk, skb, ktd)

        wv_ch = load_w(wv)
        for sc in range(SKC):
            nc.sync.dma_start(vad[sc, :, :, DH], ones[:, 0:H])
        for sc in range(SKC):
            xvt = xtp2.tile([128, KC, 128], f32r, tag="xv", name="xvt")
            nc.sync.dma_start(xvt, xv[:, sc * 128:(sc + 1) * 128]
                              .rearrange("(ko p) s -> p ko s", p=128))
            ps = psA.tile([128, 1024], f32, tag="psA", name="ps")
            for dh2 in range(2):
                ph = ps[:, dh2 * 512:(dh2 + 1) * 512]
                for kc in range(KC):
                    nc.tensor.matmul(ph, lhsT=xvt[:, kc, :],
                                     rhs=wv_ch[kc][:, dh2 * 512:(dh2 + 1) * 512],
                                     start=(kc == 0), stop=(kc == KC - 1))
            st_t = stg.tile([128, 1024], f32r, tag="stg", name="st_t")
            nc.vector.tensor_copy(st_t, ps)
            nc.sync.dma_start(
                vad[sc, :, :, 0:DH],
                st_t.rearrange("p (h d) -> p h d", h=16),
            )

        # ================= phase B: software-pipelined attention =========
        state = {}

        def emit_logits_pair(t, skc2):
            st_ = state[t]
            psl = psA.tile([128, 1024], f32, tag="psA", name="psl")
            for half in range(2):
                skc = skc2 * 2 + half
                nc.tensor.matmul(psl[:, half * 512:(half + 1) * 512],
                                 lhsT=st_["kt"][:, skc * 128:(skc + 1) * 128],
                                 rhs=st_["qt"][:],
                                 start=True, stop=True)
            pt_t = ptp.tile([128, 2, 512], f32r, tag="pt", name="pt_t")
            nc.scalar.activation(
                pt_t.rearrange("p a b -> p (a b)"), psl, Exp,
                bias=mb_sb[:, skc2 * 2:skc2 * 2 + 1], scale=0.125)
            st_["pt"].append(pt_t)

        def emit_ctx_chunk(t, skc):
            st_ = state[t]
            if skc == 0:
                st_["psc"] = psC.tile([128, 512], f32, tag="psC", name="psc")
            nc.tensor.matmul(st_["psc"][0:DH + 1, :], lhsT=st_["va"][:, skc, :],
                             rhs=st_["pt"][skc // 2][:, skc % 2, :],
                             start=(skc == 0), stop=(skc == SKC - 1))

        def emit_store(t):
            st_ = state[t]
            h, sqb = st_["h"], st_["sqb"]
            cu = stg.tile([65, 512], f32r, tag="cu", name="cu")
            with nc.allow_low_precision(reason="raw ctx rounded to f32r"):
                nc.vector.tensor_copy(cu, st_["psc"][0:DH + 1, :])
            nc.sync.dma_start(crd[h * 64:(h + 1) * 64, sqb * 512:(sqb + 1) * 512],
                              cu[0:DH, :])
            nc.sync.dma_start(dnd[t:t + 1, :], cu[DH:DH + 1, :])
            del state[t]

        cur_kt = cur_va = None
        for t in range(NT):
            h, sqb = divmod(t, 2)
            base = (h % 2) * 64
            st_ = state[t] = {"h": h, "sqb": sqb, "base": base, "pt": []}
            if sqb == 0:
                cur_kt = kts.tile([64, S], f32r, tag="kt", name="kt")
                nc.sync.dma_start(cur_kt, ktd[h * 64:(h + 1) * 64, :])
                cur_va = vas.tile([128, SKC, DH + 1], f32r, tag="va", name="va")
                nc.sync.dma_start(cur_va, vad[:, :, h, :].rearrange("sc p c -> p sc c"))
            st_["kt"], st_["va"] = cur_kt, cur_va
            qt = qts.tile([64, 512], f32r, tag="qt", name="qt")
            nc.sync.dma_start(qt,
                              qtd[h * 64:(h + 1) * 64, sqb * 512:(sqb + 1) * 512])
            st_["qt"] = qt

            for skc2 in range(SKC // 2):
                emit_logits_pair(t, skc2)
                if t >= 1:
                    emit_ctx_chunk(t - 1, skc2 * 2)
                    emit_ctx_chunk(t - 1, skc2 * 2 + 1)
            if t >= 1:
                emit_store(t - 1)

        for skc in range(SKC):
            emit_ctx_chunk(NT - 1, skc)
        emit_store(NT - 1)

        # ================= phase C: normalize + output projection =========
        wo_ch = load_w(wo)
        den_sb = consts.tile([NT, 512], f32r)
        nc.sync.dma_start(den_sb, dnd[:])
        recf = consts.tile([NT, 512], f32)
        nc.vector.reciprocal(recf, den_sb)
        rec = consts.tile([NT, 512], f32r)
        with nc.allow_low_precision(reason="softmax recip rounded to f32r"):
            nc.vector.tensor_copy(rec, recf)

        for sqb in range(2):
            rb = xtp.tile([128, KC, 512], f32r, tag="xT", name="rb")
            for kc in range(KC):
                pb = psX.tile([128, 512], f32, tag="aux", name="pb")
                nc.tensor.matmul(pb, lhsT=sel_sb[:, sqb, kc, :], rhs=rec[:],
                                 start=True, stop=True)
                with nc.allow_low_precision(reason="recip bcast rounded to f32r"):
                    nc.vector.tensor_copy(rb[:, kc, :], pb)
            for st4 in range(4):
                st8 = sqb * 4 + st4
                cT = cns2.tile([128, KC, 128], f32r, tag="cT", name="cT")
                nc.sync.dma_start(cT, crd[:, st8 * 128:(st8 + 1) * 128]
                                  .rearrange("(ko p) s -> p ko s", p=128))
                with nc.allow_low_precision(reason="normalized ctx in f32r"):
                    nc.vector.tensor_mul(out=cT, in0=cT,
                                         in1=rb[:, :, st4 * 128:(st4 + 1) * 128])
                ps = psA.tile([128, 1024], f32, tag="psA", name="ps")
                for dh2 in range(2):
                    ph = ps[:, dh2 * 512:(dh2 + 1) * 512]
                    for kc in range(KC):
                        nc.tensor.matmul(ph, lhsT=cT[:, kc, :],
                                         rhs=wo_ch[kc][:, dh2 * 512:(dh2 + 1) * 512],
                                         start=(kc == 0), stop=False)
                    nc.tensor.matmul(ph, lhsT=ones[64:65, 0:128],
                                     rhs=b3_sb[64:65, dh2 * 512:(dh2 + 1) * 512],
                                     start=False, stop=True)
                st_t = stg.tile([128, 1024], f32, tag="ost", name="ost")
                nc.vector.tensor_copy(st_t, ps)
                nc.sync.dma_start(out[st8 * 128:(st8 + 1) * 128, :], st_t)


_NC_CACHE = None


def _selector():
    s = np.zeros((NT, 2, KC, 128), np.float32)
    for kc in range(KC):
        for p in range(128):
            h = 2 * kc + p // 64
            for sqb in range(2):
                s[2 * h + sqb, sqb, kc, p] = 1.0
    return s


def kernel(query, key, value, mask, wq, bq, wk, bk, wv, bv, wo, bo):
    global _NC_CACHE
    if _NC_CACHE is None:
        _NC_CACHE = _build()
    nc = _NC_CACHE

    query = np.asarray(query, dtype=np.float32)
    key = np.asarray(key, dtype=np.float32)
    value = np.asarray(value, dtype=np.float32)
    mask = np.asarray(mask, dtype=np.float32)
    kT = [np.ascontiguousarray(key[b].T) for b in range(B)]
    vT = [np.ascontiguousarray(value[b].T) for b in range(B)]
    wo_np = np.asarray(wo, np.float32)
    # fold the V bias through the output projection: (ctx + bv) @ wo + bo
    bo_eff = (np.asarray(bo, np.float64) +
              np.asarray(bv, np.float64) @ np.asarray(wo_np, np.float64)
              ).astype(np.float32)
    b3_host = np.zeros((65, D), np.float32)
    b3_host[0] = np.asarray(bq, np.float32)
    b3_host[32] = np.asarray(bk, np.float32)
    b3_host[64] = bo_eff

    shared = {
        "wq": np.asarray(wq, np.float32), "wk": np.asarray(wk, np.float32),
        "wv": np.asarray(wv, np.float32), "wo": wo_np,
        "b3": b3_host,
        "one": np.ones((128, 512), np.float32),
        "sel": _selector(),
    }
    in_maps = []
    for core in range(N_CORES):
        b, half = divmod(core, 2)
        mbc = np.ascontiguousarray(
            (mask[b, 0, 0] * np.float32(-1e9)).reshape(S // 128, 128).T)
        in_maps.append({
            "xq": np.ascontiguousarray(query[b, half * SQ:(half + 1) * SQ].T),
            "xk": kT[b], "xv": vT[b], "mb": mbc, **shared,
        })

    res = run_bass_kernel_spmd(nc, in_maps, core_ids=list(range(N_CORES)))
    full = np.empty((B, S, D), np.float32)
    for core in range(N_CORES):
        b, half = divmod(core, 2)
        full[b, half * SQ:(half + 1) * SQ] = res.results[core]["out"]
    return full



# revision 7
# speedup vs baseline: 1.0426x; 1.0426x over previous
"""MultiHeadAttention Trainium2 kernel.

Full inputs -> full output. Sharding: 8 cores = (batch b in 0..3) x (head
group g in 0..1, 8 heads each). Each core projects Q/K/V for its head group
over all 2048 positions of batch b, runs attention for its 8 heads, applies
its half of the output projection (wo rows for its heads), and returns a
partial [2048, 1024] output. Host: full[b] = part(b,0) + part(b,1) + bias.

Everything stays in SBUF between phases (no DRAM round trips):
  phase A: project Q^T, K^T [512, 2048] (f32r, rank-1 bias matmuls) and the
           V table va [keys, head, 65] in bf16 with a ones column (so the
           softmax denominator falls out of the ctx matmul) scaled by
           exp(-1e9*mask) per key (exact mask semantics at zero cost).
  phase B: 32 software-pipelined (head, query-block) iterations:
           logits^T [sk, 512q] = K_h^T.T @ Q_h^T (f32r), P = exp(0.125*l)
           on the scalar engine -> bf16, ctx matmuls (bf16) accumulate
           [65, 512] (row 64 = denominator), DVE reciprocal, PE rank-1
           broadcast of the recips, DVE normalize-mul -> ctxn bf16 (odd
           heads take an identity matmul to shift to partitions 64..127).
  phase C: out_partial = ctxn @ wo (bf16 x bf16), f32 partials to DRAM.
"""

import numpy as np
import ml_dtypes

import concourse.bass as bass
import concourse.mybir as mybir
import concourse.tile as tile
from concourse import bacc
from concourse.bass_utils import run_bass_kernel_spmd

f32 = mybir.dt.float32
f32r = mybir.dt.float32r
bf16 = mybir.dt.bfloat16
np_bf16 = ml_dtypes.bfloat16

B, S, D, H, DH = 4, 2048, 1024, 16, 64
HG = H // 2          # 8 heads per core
DG = HG * DH         # 512 projection cols per core
N_CORES = 8
Exp = mybir.ActivationFunctionType.Exp

KC = D // 128        # 8 contraction chunks over d_model
CC = DG // 128       # 4 chunks over the head-group dim
SKC = S // 128       # 16 key chunks
NT = HG * 4          # 32 pipelined iterations (head, 512-query block)


def _build():
    nc = bacc.Bacc(None, target_bir_lowering=False)

    xq = nc.dram_tensor("xq", [D, S], f32r, kind="ExternalInput")   # query^T
    xk = nc.dram_tensor("xk", [D, S], f32r, kind="ExternalInput")   # key^T
    xv = nc.dram_tensor("xv", [D, S], f32r, kind="ExternalInput")   # value^T
    wq = nc.dram_tensor("wq", [D, DG], f32r, kind="ExternalInput")
    wk = nc.dram_tensor("wk", [D, DG], f32r, kind="ExternalInput")
    wv = nc.dram_tensor("wv", [D, DG], f32r, kind="ExternalInput")
    wo = nc.dram_tensor("wo", [DG, D], bf16, kind="ExternalInput")
    b2 = nc.dram_tensor("b2", [33, DG], f32r, kind="ExternalInput")  # bq row 0, bk row 32
    one = nc.dram_tensor("one", [33, 512], f32r, kind="ExternalInput")
    emask8 = nc.dram_tensor("emask8", [128, SKC, HG], bf16, kind="ExternalInput")
    emaskf = nc.dram_tensor("emaskf", [128, SKC], f32, kind="ExternalInput")
    ident = nc.dram_tensor("ident", [64, 128], f32r, kind="ExternalInput")
    out = nc.dram_tensor("out", [S, D], f32, kind="ExternalOutput")

    with tile.TileContext(nc) as tc:
        _emit(nc, tc, xq, xk, xv, wq, wk, wv, wo, b2, one, emask8, emaskf,
              ident, out)
    nc.finalize()
    return nc


def _emit(nc, tc, xq, xk, xv, wq, wk, wv, wo, b2, one, emask8, emaskf,
          ident, out):
    from contextlib import ExitStack

    with ExitStack() as ctx:
        consts = ctx.enter_context(tc.tile_pool(name="consts", bufs=1))
        wpool = ctx.enter_context(tc.tile_pool(name="wpool", bufs=2))
        xtp = ctx.enter_context(tc.tile_pool(name="xtp", bufs=2))
        big = ctx.enter_context(tc.tile_pool(name="big", bufs=1))
        ptp = ctx.enter_context(tc.tile_pool(name="ptp", bufs=10))
        tmp = ctx.enter_context(tc.tile_pool(name="tmp", bufs=1))
        rcp = ctx.enter_context(tc.tile_pool(name="rcp", bufs=1))
        stg = ctx.enter_context(tc.tile_pool(name="stg", bufs=2))
        psl = ctx.enter_context(tc.tile_pool(name="psl", bufs=2, space="PSUM"))
        psc = ctx.enter_context(tc.tile_pool(name="psc", bufs=2, space="PSUM"))
        psx = ctx.enter_context(tc.tile_pool(name="psx", bufs=2, space="PSUM"))

        b2_sb = consts.tile([33, DG], f32r)
        nc.sync.dma_start(b2_sb, b2[:])
        ones = consts.tile([33, 512], f32r)
        nc.sync.dma_start(ones, one[:])
        em_sb = consts.tile([128, SKC], f32)
        nc.sync.dma_start(em_sb, emaskf[:])
        id_sb = consts.tile([64, 128], f32r)
        nc.sync.dma_start(id_sb, ident[:])

        qt_sb = big.tile([128, CC, S], f32r)        # Q^T: head h at [(h%2)*64, h//2]
        kt_sb = big.tile([128, CC, S], f32r)        # K^T: same layout
        va_sb = big.tile([128, SKC, HG, DH + 1], bf16)  # [v*em, em] per key/head
        cx_sb = big.tile([128, CC, S], bf16)        # normalized ctx^T

        # ones column of va = exp(-1e9*mask) per key
        nc.sync.dma_start(va_sb[:, :, :, DH], emask8[:])

        # ================= phase A: projections =================
        def project_T(w_dram, brow, x_dram, dst_sb):
            """Q^T / K^T [512, 2048] = w_g^T @ x^T, bias via rank-1 matmul."""
            wt = wpool.tile([128, KC, DG], f32r, tag="w", name="wt")
            nc.sync.dma_start(wt, w_dram[:].rearrange("(ko p) c -> p ko c", p=128))
            for blk in range(4):
                xT = xtp.tile([128, KC, 512], f32r, tag="xT", name="xT")
                nc.sync.dma_start(xT, x_dram[:, blk * 512:(blk + 1) * 512]
                                  .rearrange("(ko p) s -> p ko s", p=128))
                for cc in range(CC):
                    ps = psx.tile([128, 512], f32, tag="psx", name="ps")
                    for kc in range(KC):
                        nc.tensor.matmul(ps, lhsT=wt[:, kc, cc * 128:(cc + 1) * 128],
                                         rhs=xT[:, kc, :],
                                         start=(kc == 0), stop=False)
                    nc.tensor.matmul(ps, lhsT=b2_sb[brow:brow + 1, cc * 128:(cc + 1) * 128],
                                     rhs=ones[brow:brow + 1, 0:512],
                                     start=False, stop=True)
                    with nc.allow_low_precision(reason="proj rounded to f32r"):
                        nc.vector.tensor_copy(
                            dst_sb[:, cc, blk * 512:(blk + 1) * 512], ps)

        project_T(wq, 0, xq, qt_sb)
        project_T(wk, 32, xk, kt_sb)

        # V: [keys, 8h*64] scaled by emask per key, interleaved into va
        wvt = wpool.tile([128, KC, DG], f32r, tag="w", name="wvt")
        nc.sync.dma_start(wvt, wv[:].rearrange("(ko p) c -> p ko c", p=128))
        for sc in range(SKC):
            xvt = xtp.tile([128, KC, 128], f32r, tag="xT", name="xvt")
            nc.sync.dma_start(xvt, xv[:, sc * 128:(sc + 1) * 128]
                              .rearrange("(ko p) s -> p ko s", p=128))
            ps = psx.tile([128, 512], f32, tag="psx", name="ps")
            for kc in range(KC):
                nc.tensor.matmul(ps, lhsT=xvt[:, kc, :], rhs=wvt[:, kc, :],
                                 start=(kc == 0), stop=(kc == KC - 1))
            with nc.allow_low_precision(reason="va in bf16"):
                nc.vector.tensor_scalar_mul(
                    va_sb[:, sc, :, 0:DH],
                    ps.rearrange("p (h d) -> p h d", h=HG),
                    em_sb[:, sc:sc + 1])

        # ================= phase B: pipelined attention =================
        state = {}

        def emit_logits_pair(t, kcp):
            st_ = state[t]
            h, sqb = st_["h"], st_["sqb"]
            hp, hcc = (h % 2) * 64, h // 2
            ps_ = psl.tile([128, 1024], f32, tag="psl", name="psl")
            for half in range(2):
                skc = kcp * 2 + half
                nc.tensor.matmul(ps_[:, half * 512:(half + 1) * 512],
                                 lhsT=kt_sb[hp:hp + 64, hcc,
                                            skc * 128:(skc + 1) * 128],
                                 rhs=qt_sb[hp:hp + 64, hcc,
                                           sqb * 512:(sqb + 1) * 512],
                                 start=True, stop=True)
            pt = ptp.tile([128, 2, 512], bf16, tag="pt", name="pt")
            nc.scalar.activation(pt.rearrange("p a b -> p (a b)"), ps_, Exp,
                                 scale=0.125)
            st_["pt"].append(pt)

        def emit_ctx_chunk(t, skc):
            st_ = state[t]
            if skc == 0:
                st_["psc"] = psc.tile([128, 512], f32, tag="psc", name="psc")
            nc.tensor.matmul(st_["psc"][0:DH + 1, :],
                             lhsT=va_sb[:, skc, st_["h"], :],
                             rhs=st_["pt"][skc // 2][:, skc % 2, :],
                             start=(skc == 0), stop=(skc == SKC - 1))

        def emit_norm(t):
            st_ = state[t]
            h, sqb = st_["h"], st_["sqb"]
            hcc, odd = h // 2, h % 2
            cu = stg.tile([DH + 1, 512], f32, tag="cu", name="cu")
            nc.vector.tensor_copy(cu, st_["psc"][0:DH + 1, :])
            rec = rcp.tile([1, 512], f32r, tag="rec", name="rec")
            with nc.allow_low_precision(reason="recip rounded to f32r"):
                nc.vector.reciprocal(rec, cu[DH:DH + 1, :])
            bc = psx.tile([128, 512], f32, tag="psx", name="bc")
            nc.tensor.matmul(bc[0:64, :], lhsT=ones[0:1, 0:64], rhs=rec[:],
                             start=True, stop=True)
            dst = cx_sb[64 * odd:64 * odd + 64, hcc, sqb * 512:(sqb + 1) * 512]
            with nc.allow_low_precision(reason="ctxn in bf16"):
                if not odd:
                    nc.vector.tensor_mul(out=dst, in0=cu[0:DH, :],
                                         in1=bc[0:64, :])
                else:
                    tm = tmp.tile([64, 512], f32r, tag="tmp", name="tm")
                    nc.vector.tensor_mul(out=tm, in0=cu[0:DH, :],
                                         in1=bc[0:64, :])
                    sh = psx.tile([128, 512], f32, tag="psx", name="sh")
                    nc.tensor.matmul(sh, lhsT=id_sb[:], rhs=tm[:],
                                     start=True, stop=True)
                    nc.vector.tensor_copy(dst, sh[64:128, :])
            del state[t]

        for t in range(NT):
            h, sqb = divmod(t, 4)
            state[t] = {"h": h, "sqb": sqb, "pt": []}
            for kcp in range(SKC // 2):
                emit_logits_pair(t, kcp)
                if t >= 1:
                    emit_ctx_chunk(t - 1, kcp * 2)
                    emit_ctx_chunk(t - 1, kcp * 2 + 1)
            if t >= 1:
                emit_norm(t - 1)
        for skc in range(SKC):
            emit_ctx_chunk(NT - 1, skc)
        emit_norm(NT - 1)

        # ================= phase C: output projection =================
        wot = wpool.tile([128, CC, D], bf16, tag="w", name="wot")
        nc.sync.dma_start(wot, wo[:].rearrange("(co p) c -> p co c", p=128))
        for st8 in range(SKC):
            ot = stg.tile([128, 1024], f32, tag="ost", name="ot")
            for half in range(2):
                ps = psx.tile([128, 512], f32, tag="psx", name="ps")
                for cc in range(CC):
                    nc.tensor.matmul(ps,
                                     lhsT=cx_sb[:, cc, st8 * 128:(st8 + 1) * 128],
                                     rhs=wot[:, cc, half * 512:(half + 1) * 512],
                                     start=(cc == 0), stop=(cc == CC - 1))
                nc.vector.tensor_copy(ot[:, half * 512:(half + 1) * 512], ps)
            nc.sync.dma_start(out[st8 * 128:(st8 + 1) * 128, :], ot)


_NC_CACHE = None


def kernel(query, key, value, mask, wq, bq, wk, bk, wv, bv, wo, bo):
    global _NC_CACHE
    if _NC_CACHE is None:
        _NC_CACHE = _build()
    nc = _NC_CACHE

    query = np.asarray(query, dtype=np.float32)
    key = np.asarray(key, dtype=np.float32)
    value = np.asarray(value, dtype=np.float32)
    mask = np.asarray(mask, dtype=np.float32)
    wq_np = np.asarray(wq, np.float32)
    wk_np = np.asarray(wk, np.float32)
    wv_np = np.asarray(wv, np.float32)
    wo_np = np.asarray(wo, np.float32)
    bq_np = np.asarray(bq, np.float32)
    bk_np = np.asarray(bk, np.float32)
    # fold bv and bo through the output projection (added on host at the end)
    bias_out = (np.asarray(bo, np.float64) +
                np.asarray(bv, np.float64) @ np.asarray(wo_np, np.float64)
                ).astype(np.float32)

    xT = {}
    for b in range(B):
        xT[b] = (np.ascontiguousarray(query[b].T),
                 np.ascontiguousarray(key[b].T),
                 np.ascontiguousarray(value[b].T))
    shared_g = []
    for g in range(2):
        cols = slice(DG * g, DG * (g + 1))
        b2_host = np.zeros((33, DG), np.float32)
        b2_host[0] = bq_np[cols]
        b2_host[32] = bk_np[cols]
        shared_g.append({
            "wq": np.ascontiguousarray(wq_np[:, cols]),
            "wk": np.ascontiguousarray(wk_np[:, cols]),
            "wv": np.ascontiguousarray(wv_np[:, cols]),
            "wo": np.ascontiguousarray(wo_np[cols, :]).astype(np_bf16),
            "b2": np.ascontiguousarray(b2_host),
        })
    one_host = np.ones((33, 512), np.float32)
    id_host = np.concatenate([np.zeros((64, 64), np.float32),
                              np.eye(64, dtype=np.float32)], axis=1)

    in_maps = []
    for core in range(N_CORES):
        b, g = divmod(core, 2)
        em = np.exp(mask[b, 0, 0] * np.float32(-1e9)).astype(np.float32)
        emc = np.ascontiguousarray(em.reshape(SKC, 128).T)   # [128, SKC]
        em8 = np.ascontiguousarray(
            np.repeat(emc[:, :, None], HG, axis=2)).astype(np_bf16)
        in_maps.append({
            "xq": xT[b][0], "xk": xT[b][1], "xv": xT[b][2],
            "emask8": em8, "emaskf": emc,
            "one": one_host, "ident": id_host,
            **shared_g[g],
        })

    res = run_bass_kernel_spmd(nc, in_maps, core_ids=list(range(N_CORES)))
    full = np.empty((B, S, D), np.float32)
    for b in range(B):
        full[b] = res.results[2 * b]["out"]
        full[b] += res.results[2 * b + 1]["out"]
        full[b] += bias_out
    return full


# revision 8
# speedup vs baseline: 1.0834x; 1.0392x over previous
"""MultiHeadAttention Trainium2 kernel.

Full inputs -> full output. Sharding: 8 cores = (batch b in 0..3) x (head
group g in 0..1, 8 heads each). Each core projects Q/K/V for its head group
over all 2048 positions of batch b, runs attention for its 8 heads, applies
its half of the output projection (wo rows for its heads), and returns a
partial [2048, 1024] output. Host: full[b] = part(b,0) + part(b,1) + bias.

Everything stays in SBUF between phases (no DRAM round trips):
  phase A: project Q^T, K^T [512, 2048] (f32r, rank-1 bias matmuls) and the
           V table va [keys, head, 65] in bf16 with a ones column (so the
           softmax denominator falls out of the ctx matmul) scaled by
           exp(-1e9*mask) per key (exact mask semantics at zero cost).
  phase B: 32 software-pipelined (head, query-block) iterations:
           logits^T [sk, 512q] = K_h^T.T @ Q_h^T (f32r), P = exp(0.125*l)
           on the scalar engine -> bf16, ctx matmuls (bf16) accumulate
           [65, 512] (row 64 = denominator), DVE reciprocal, PE rank-1
           broadcast of the recips, DVE normalize-mul -> ctxn bf16 (odd
           heads take an identity matmul to shift to partitions 64..127).
  phase C: out_partial = ctxn @ wo (bf16 x bf16), f32 partials to DRAM.
"""

import numpy as np
import ml_dtypes

import concourse.bass as bass
import concourse.mybir as mybir
import concourse.tile as tile
from concourse import bacc
from concourse.bass_utils import run_bass_kernel_spmd

f32 = mybir.dt.float32
f32r = mybir.dt.float32r
bf16 = mybir.dt.bfloat16
np_bf16 = ml_dtypes.bfloat16

B, S, D, H, DH = 4, 2048, 1024, 16, 64
HG = H // 2          # 8 heads per core
DG = HG * DH         # 512 projection cols per core
N_CORES = 8
Exp = mybir.ActivationFunctionType.Exp

KC = D // 128        # 8 contraction chunks over d_model
CC = DG // 128       # 4 chunks over the head-group dim
SKC = S // 128       # 16 key chunks
NT = HG * 4          # 32 pipelined iterations (head, 512-query block)


def _build():
    nc = bacc.Bacc(None, target_bir_lowering=False)

    xq = nc.dram_tensor("xq", [D, S], f32r, kind="ExternalInput")   # query^T
    xk = nc.dram_tensor("xk", [D, S], f32r, kind="ExternalInput")   # key^T
    xv = nc.dram_tensor("xv", [D, S], f32r, kind="ExternalInput")   # value^T
    wq = nc.dram_tensor("wq", [D, DG], f32r, kind="ExternalInput")
    wk = nc.dram_tensor("wk", [D, DG], f32r, kind="ExternalInput")
    wv = nc.dram_tensor("wv", [D, DG], f32r, kind="ExternalInput")
    wo = nc.dram_tensor("wo", [DG, D], bf16, kind="ExternalInput")
    b2 = nc.dram_tensor("b2", [33, DG], f32r, kind="ExternalInput")  # bq row 0, bk row 32
    one = nc.dram_tensor("one", [33, 512], f32r, kind="ExternalInput")
    emask8 = nc.dram_tensor("emask8", [128, SKC, HG], bf16, kind="ExternalInput")
    emaskf = nc.dram_tensor("emaskf", [128, SKC], f32, kind="ExternalInput")
    ident = nc.dram_tensor("ident", [64, 128], bf16, kind="ExternalInput")
    oneb = nc.dram_tensor("oneb", [1, 64], bf16, kind="ExternalInput")
    out = nc.dram_tensor("out", [S, D], f32, kind="ExternalOutput")

    with tile.TileContext(nc) as tc:
        _emit(nc, tc, xq, xk, xv, wq, wk, wv, wo, b2, one, emask8, emaskf,
              ident, oneb, out)
    nc.finalize()
    return nc


def _emit(nc, tc, xq, xk, xv, wq, wk, wv, wo, b2, one, emask8, emaskf,
          ident, oneb, out):
    from contextlib import ExitStack

    with ExitStack() as ctx:
        consts = ctx.enter_context(tc.tile_pool(name="consts", bufs=1))
        wpool = ctx.enter_context(tc.tile_pool(name="wpool", bufs=2))
        xtp = ctx.enter_context(tc.tile_pool(name="xtp", bufs=2))
        big = ctx.enter_context(tc.tile_pool(name="big", bufs=1))
        ptp = ctx.enter_context(tc.tile_pool(name="ptp", bufs=12))
        tmp = ctx.enter_context(tc.tile_pool(name="tmp", bufs=1))
        rcp = ctx.enter_context(tc.tile_pool(name="rcp", bufs=1))
        stg = ctx.enter_context(tc.tile_pool(name="stg", bufs=2))
        psl = ctx.enter_context(tc.tile_pool(name="psl", bufs=2, space="PSUM"))
        psc = ctx.enter_context(tc.tile_pool(name="psc", bufs=2, space="PSUM"))
        psx = ctx.enter_context(tc.tile_pool(name="psx", bufs=2, space="PSUM"))

        b2_sb = consts.tile([33, DG], f32r)
        nc.sync.dma_start(b2_sb, b2[:])
        ones = consts.tile([33, 512], f32r)
        nc.sync.dma_start(ones, one[:])
        em_sb = consts.tile([128, SKC], f32)
        nc.sync.dma_start(em_sb, emaskf[:])
        id_sb = consts.tile([64, 128], bf16)
        nc.sync.dma_start(id_sb, ident[:])
        onesb = consts.tile([1, 64], bf16)
        nc.sync.dma_start(onesb, oneb[:])

        qt_sb = big.tile([128, CC, S], bf16)        # Q^T: head h at [(h%2)*64, h//2]
        kt_sb = big.tile([128, CC, S], bf16)        # K^T: same layout
        va_sb = big.tile([128, SKC, HG, DH + 1], bf16)  # [v*em, em] per key/head
        cx_sb = big.tile([128, CC, S], bf16)        # normalized ctx^T

        # ones column of va = exp(-1e9*mask) per key
        nc.sync.dma_start(va_sb[:, :, :, DH], emask8[:])

        # ================= phase A: projections =================
        def project_T(w_dram, brow, x_dram, dst_sb):
            """Q^T / K^T [512, 2048] = w_g^T @ x^T, bias via rank-1 matmul."""
            wt = wpool.tile([128, KC, DG], f32r, tag="w", name="wt")
            nc.sync.dma_start(wt, w_dram[:].rearrange("(ko p) c -> p ko c", p=128))
            for blk in range(4):
                xT = xtp.tile([128, KC, 512], f32r, tag="xT", name="xT")
                nc.sync.dma_start(xT, x_dram[:, blk * 512:(blk + 1) * 512]
                                  .rearrange("(ko p) s -> p ko s", p=128))
                for cc in range(CC):
                    ps = psx.tile([128, 512], f32, tag="psx", name="ps")
                    for kc in range(KC):
                        nc.tensor.matmul(ps, lhsT=wt[:, kc, cc * 128:(cc + 1) * 128],
                                         rhs=xT[:, kc, :],
                                         start=(kc == 0), stop=False)
                    nc.tensor.matmul(ps, lhsT=b2_sb[brow:brow + 1, cc * 128:(cc + 1) * 128],
                                     rhs=ones[brow:brow + 1, 0:512],
                                     start=False, stop=True)
                    with nc.allow_low_precision(reason="proj rounded to f32r"):
                        nc.vector.tensor_copy(
                            dst_sb[:, cc, blk * 512:(blk + 1) * 512], ps)

        project_T(wq, 0, xq, qt_sb)
        project_T(wk, 32, xk, kt_sb)

        # V: [keys, 8h*64] scaled by emask per key, interleaved into va
        wvt = wpool.tile([128, KC, DG], f32r, tag="w", name="wvt")
        nc.sync.dma_start(wvt, wv[:].rearrange("(ko p) c -> p ko c", p=128))
        for sc in range(SKC):
            xvt = xtp.tile([128, KC, 128], f32r, tag="xT", name="xvt")
            nc.sync.dma_start(xvt, xv[:, sc * 128:(sc + 1) * 128]
                              .rearrange("(ko p) s -> p ko s", p=128))
            ps = psx.tile([128, 512], f32, tag="psx", name="ps")
            for kc in range(KC):
                nc.tensor.matmul(ps, lhsT=xvt[:, kc, :], rhs=wvt[:, kc, :],
                                 start=(kc == 0), stop=(kc == KC - 1))
            with nc.allow_low_precision(reason="va in bf16"):
                nc.vector.tensor_scalar_mul(
                    va_sb[:, sc, :, 0:DH],
                    ps.rearrange("p (h d) -> p h d", h=HG),
                    em_sb[:, sc:sc + 1])

        # ================= phase B: pipelined attention =================
        state = {}

        def emit_logits_pair(t, kcp):
            st_ = state[t]
            h, sqb = st_["h"], st_["sqb"]
            hp, hcc = (h % 2) * 64, h // 2
            ps_ = psl.tile([128, 1024], f32, tag="psl", name="psl")
            for half in range(2):
                skc = kcp * 2 + half
                nc.tensor.matmul(ps_[:, half * 512:(half + 1) * 512],
                                 lhsT=kt_sb[hp:hp + 64, hcc,
                                            skc * 128:(skc + 1) * 128],
                                 rhs=qt_sb[hp:hp + 64, hcc,
                                           sqb * 512:(sqb + 1) * 512],
                                 start=True, stop=True)
            pt = ptp.tile([128, 2, 512], bf16, tag="pt", name="pt")
            nc.scalar.activation(pt.rearrange("p a b -> p (a b)"), ps_, Exp,
                                 scale=0.125)
            st_["pt"].append(pt)

        def emit_ctx_chunk(t, skc):
            st_ = state[t]
            if skc == 0:
                st_["psc"] = psc.tile([128, 512], f32, tag="psc", name="psc")
            nc.tensor.matmul(st_["psc"][0:DH + 1, :],
                             lhsT=va_sb[:, skc, st_["h"], :],
                             rhs=st_["pt"][skc // 2][:, skc % 2, :],
                             start=(skc == 0), stop=(skc == SKC - 1))

        def emit_norm(t):
            st_ = state[t]
            h, sqb = st_["h"], st_["sqb"]
            hcc, odd = h // 2, h % 2
            cu = stg.tile([DH + 1, 512], f32, tag="cu", name="cu")
            nc.vector.tensor_copy(cu, st_["psc"][0:DH + 1, :])
            rec = rcp.tile([1, 512], bf16, tag="rec", name="rec")
            with nc.allow_low_precision(reason="recip rounded to f32r"):
                nc.vector.reciprocal(rec, cu[DH:DH + 1, :])
            bc = psx.tile([128, 512], f32, tag="psx", name="bc")
            nc.tensor.matmul(bc[0:64, :], lhsT=onesb[0:1, :], rhs=rec[:],
                             start=True, stop=True)
            dst = cx_sb[64 * odd:64 * odd + 64, hcc, sqb * 512:(sqb + 1) * 512]
            with nc.allow_low_precision(reason="ctxn in bf16"):
                if not odd:
                    nc.vector.tensor_mul(out=dst, in0=cu[0:DH, :],
                                         in1=bc[0:64, :])
                else:
                    tm = tmp.tile([64, 512], bf16, tag="tmp", name="tm")
                    nc.vector.tensor_mul(out=tm, in0=cu[0:DH, :],
                                         in1=bc[0:64, :])
                    sh = psx.tile([128, 512], f32, tag="psx", name="sh")
                    nc.tensor.matmul(sh, lhsT=id_sb[:], rhs=tm[:],
                                     start=True, stop=True)
                    nc.vector.tensor_copy(dst, sh[64:128, :])
            del state[t]

        for t in range(NT):
            h, sqb = divmod(t, 4)
            state[t] = {"h": h, "sqb": sqb, "pt": []}
            for kcp in range(SKC // 2):
                emit_logits_pair(t, kcp)
                if t >= 1:
                    emit_ctx_chunk(t - 1, kcp * 2)
                    emit_ctx_chunk(t - 1, kcp * 2 + 1)
            if t >= 1:
                emit_norm(t - 1)
        for skc in range(SKC):
            emit_ctx_chunk(NT - 1, skc)
        emit_norm(NT - 1)

        # ================= phase C: output projection =================
        wot = wpool.tile([128, CC, D], bf16, tag="w", name="wot")
        nc.sync.dma_start(wot, wo[:].rearrange("(co p) c -> p co c", p=128))
        for st8 in range(SKC):
            ot = stg.tile([128, 1024], f32, tag="ost", name="ot")
            for half in range(2):
                ps = psx.tile([128, 512], f32, tag="psx", name="ps")
                for cc in range(CC):
                    nc.tensor.matmul(ps,
                                     lhsT=cx_sb[:, cc, st8 * 128:(st8 + 1) * 128],
                                     rhs=wot[:, cc, half * 512:(half + 1) * 512],
                                     start=(cc == 0), stop=(cc == CC - 1))
                nc.vector.tensor_copy(ot[:, half * 512:(half + 1) * 512], ps)
            nc.sync.dma_start(out[st8 * 128:(st8 + 1) * 128, :], ot)


_NC_CACHE = None


def kernel(query, key, value, mask, wq, bq, wk, bk, wv, bv, wo, bo):
    global _NC_CACHE
    if _NC_CACHE is None:
        _NC_CACHE = _build()
    nc = _NC_CACHE

    query = np.asarray(query, dtype=np.float32)
    key = np.asarray(key, dtype=np.float32)
    value = np.asarray(value, dtype=np.float32)
    mask = np.asarray(mask, dtype=np.float32)
    wq_np = np.asarray(wq, np.float32)
    wk_np = np.asarray(wk, np.float32)
    wv_np = np.asarray(wv, np.float32)
    wo_np = np.asarray(wo, np.float32)
    bq_np = np.asarray(bq, np.float32)
    bk_np = np.asarray(bk, np.float32)
    # fold bv and bo through the output projection (added on host at the end)
    bias_out = (np.asarray(bo, np.float64) +
                np.asarray(bv, np.float64) @ np.asarray(wo_np, np.float64)
                ).astype(np.float32)

    xT = {}
    for b in range(B):
        xT[b] = (np.ascontiguousarray(query[b].T),
                 np.ascontiguousarray(key[b].T),
                 np.ascontiguousarray(value[b].T))
    shared_g = []
    for g in range(2):
        cols = slice(DG * g, DG * (g + 1))
        b2_host = np.zeros((33, DG), np.float32)
        b2_host[0] = bq_np[cols]
        b2_host[32] = bk_np[cols]
        shared_g.append({
            "wq": np.ascontiguousarray(wq_np[:, cols]),
            "wk": np.ascontiguousarray(wk_np[:, cols]),
            "wv": np.ascontiguousarray(wv_np[:, cols]),
            "wo": np.ascontiguousarray(wo_np[cols, :]).astype(np_bf16),
            "b2": np.ascontiguousarray(b2_host),
        })
    one_host = np.ones((33, 512), np.float32)
    id_host = np.concatenate([np.zeros((64, 64), np.float32),
                              np.eye(64, dtype=np.float32)], axis=1).astype(np_bf16)
    oneb_host = np.ones((1, 64), np_bf16)

    in_maps = []
    for core in range(N_CORES):
        b, g = divmod(core, 2)
        em = np.exp(mask[b, 0, 0] * np.float32(-1e9)).astype(np.float32)
        emc = np.ascontiguousarray(em.reshape(SKC, 128).T)   # [128, SKC]
        em8 = np.ascontiguousarray(
            np.repeat(emc[:, :, None], HG, axis=2)).astype(np_bf16)
        in_maps.append({
            "xq": xT[b][0], "xk": xT[b][1], "xv": xT[b][2],
            "emask8": em8, "emaskf": emc,
            "one": one_host, "ident": id_host, "oneb": oneb_host,
            **shared_g[g],
        })

    res = run_bass_kernel_spmd(nc, in_maps, core_ids=list(range(N_CORES)))
    full = np.empty((B, S, D), np.float32)
    for b in range(B):
        full[b] = res.results[2 * b]["out"]
        full[b] += res.results[2 * b + 1]["out"]
        full[b] += bias_out
    return full
